# revision 15
# baseline (speedup 1.0000x reference)
"""DGMRF 2-layer GNN message passing on 8 TRN2 NeuronCores.

Strategy (per layer, per core):
  - Nodes sharded by dst: core c owns dst nodes [12500c, 12500(c+1)).
  - Aggregation aggr[:, d] = sum_{e: dst=d} x[:, src_e] done on-device via
    GPSIMD ap_gather (SBUF free-dim gather, per-16-partition-group index
    lists) + DVE tensor_reduce over per-dst slot grids.
  - x is chunked into 4 source chunks of 25000 nodes (int16 index limit);
    per chunk, per exact-degree class K, a padded [Npad, K] grid of
    chunk-local src indices is gathered and reduced; per-chunk results are
    reordered into natural dst order with a second (small) ap_gather and
    accumulated.
  - Per-node factors f1/f2 (degree weights x layer scalars) are computed on
    host; out = f1*x_self + f2*aggr + bias.
  - One compiled SPMD program, run once per layer; host re-assembles the
    full x between layers (the cross-core "halo exchange").
"""
import numpy as np

import concourse.bass as bass
import concourse.mybir as mybir
import concourse.tile as tile
from concourse import bacc
from concourse.bass_utils import run_bass_kernel_spmd

N_NODES = 100000
N_CORES = 8
NPC = N_NODES // N_CORES          # 12500 dst nodes per core
G = 8                             # partition groups per core
GW_REAL = 1563                    # dst nodes per group (last group short)
GW = 1568                         # padded group width (mult of 16 for idx wrap)
NPC_PAD = G * GW_REAL             # 12504
CHUNK = 25000
N_CHUNKS = N_NODES // CHUNK       # 4
TBL = CHUNK + 1                   # table width incl zero dummy col
DUMMY = CHUNK                     # dummy (zero) column index


def _round_up(x, m):
    return (x + m - 1) // m * m


def _node_group(j):
    """group id for core-local padded dst position j (0..NPC_PAD)."""
    return np.minimum(j // GW_REAL, G - 1)


def _wrap_idx(flat):
    """[G, Q] flat per-group index lists -> wrapped [128, Q//16] int16.

    ap_gather consumes per-group lists interleaved: list position q lives at
    partition 16g + q%16, column q//16."""
    Gn, Q = flat.shape
    assert Q % 16 == 0
    w = flat.reshape(Gn, Q // 16, 16).transpose(0, 2, 1)  # [G, 16, Q//16]
    return w.reshape(Gn * 16, Q // 16).astype(np.int16)


def make_plan(edge_index):
    """Host-side planning: class structure (global) + per-core index data."""
    src = np.asarray(edge_index[0], dtype=np.int64)
    dst = np.asarray(edge_index[1], dtype=np.int64)
    E = src.shape[0]

    chunk_of = src // CHUNK
    # per (dst, chunk) degree
    cnt = np.bincount(dst * N_CHUNKS + chunk_of, minlength=N_NODES * N_CHUNKS)
    cnt = cnt.reshape(N_NODES, N_CHUNKS)

    # edges sorted by (dst, chunk): per (dst, chunk) contiguous src runs
    order = np.lexsort((chunk_of, dst))
    s_src_local = (src[order] % CHUNK).astype(np.int32)
    starts = np.zeros(N_NODES * N_CHUNKS + 1, np.int64)
    np.cumsum(cnt.reshape(-1), out=starts[1:])

    # core-local padded dst table: pos (c, j) -> node id or -1 (pad)
    # groups are contiguous slices of width GW_REAL
    core_node = np.full((N_CORES, NPC_PAD), -1, np.int64)
    for c in range(N_CORES):
        core_node[c, :NPC] = np.arange(NPC * c, NPC * (c + 1))
    grp_of_j = _node_group(np.arange(NPC_PAD))

    # pass 1 per chunk: class structure (global across cores/groups),
    # per-core grid flats and tempcol maps
    chunk_info = []
    for cc in range(N_CHUNKS):
        deg = np.zeros((N_CORES, NPC_PAD), np.int64)
        valid = core_node >= 0
        deg[valid] = cnt[core_node[valid], cc]

        kmax = int(deg.max())
        # global per-(core,group,K) counts
        counts = np.zeros((N_CORES, G, kmax + 1), np.int64)
        for c in range(N_CORES):
            key = grp_of_j * (kmax + 1) + deg[c]
            bc = np.bincount(key, minlength=G * (kmax + 1))
            counts[c] = bc.reshape(G, kmax + 1)
        gmax = counts.max(axis=(0, 1))  # [kmax+1] max members per group

        # kept classes: merge small ones upward
        kept = []
        for K in range(1, kmax + 1):
            if gmax[K] == 0:
                continue
            kept.append(K)
        # merge: keep K if its max count >= 48, else merge into next kept
        merged = []
        for K in kept:
            merged.append(K)
        # assignment class of each deg d (>0): smallest kept >= d
        keep_mask = [False] * (kmax + 2)
        sel = []
        for K in kept:
            if gmax[K] >= 48 or K == kept[-1]:
                sel.append(K)
        if not sel or sel[-1] < kmax:
            sel.append(kmax)
        sel = sorted(set(sel))
        cls_of = np.zeros(kmax + 1, np.int64)  # deg -> class K (0 for deg 0)
        si = 0
        for d in range(1, kmax + 1):
            while sel[si] < d:
                si += 1
            cls_of[d] = sel[si]

        # per class: Npad (global), temp offset
        cls_list = []
        off = 0
        npad_of = {}
        for K in sel:
            mx = 0
            for c in range(N_CORES):
                cdeg = deg[c]
                mask = (cdeg > 0) & (cls_of[cdeg] == K)
                bc = np.bincount(grp_of_j[mask], minlength=G)
                mx = max(mx, int(bc.max()))
            # classes are merged into one gather per chunk, so only the batch
            # total needs idx alignment; npad just needs a zero pad cell
            npad = _round_up(mx + 1, 2)
            npad_of[K] = (npad, off)
            cls_list.append((K, npad, off))
            off += npad
        WT = off
        zero_cell = cls_list[0][2] + cls_list[0][1] - 1  # a guaranteed 0 cell

        q_real = sum(npad * K for K, npad, _ in cls_list)

        flats = []     # per core: [G, q_real] grid content (table cols)
        tempcols = []  # per core: [NPC_PAD] scratch col of each dst
        for c in range(N_CORES):
            cdeg = deg[c]
            flat_parts = []
            tempcol = np.full(NPC_PAD, zero_cell, np.int64)
            for K, npad, offK in cls_list:
                mask = (cdeg > 0) & (cls_of[cdeg] == K)
                js = np.nonzero(mask)[0]
                gs = grp_of_j[js]
                # rank via cumcount per group (js ascending => natural order)
                r = np.zeros(len(js), np.int64)
                for g in range(G):
                    m = gs == g
                    r[m] = np.arange(m.sum())
                tempcol[js] = offK + r
                # fill srcs: node n, chunk cc: run starts[n*4+cc], length deg
                nodes = core_node[c, js]
                st = starts[nodes * N_CHUNKS + cc]
                dg = cdeg[js]
                A = np.full((G * npad, K), DUMMY, np.int32)
                if len(js):
                    kk = np.arange(K)[None, :]
                    valid = kk < dg[:, None]
                    pos = np.minimum(st[:, None] + kk, len(s_src_local) - 1)
                    vals = s_src_local[pos]
                    A[gs * npad + r] = np.where(valid, vals, DUMMY)
                flat_parts.append(A.reshape(G, npad * K))
            flats.append(np.concatenate(flat_parts, axis=1))
            tempcols.append(tempcol)

        coffs = []
        so = 0
        for K, npad, offK in cls_list:
            coffs.append((K, npad, offK, so))
            so += npad * K
        assert so == q_real
        chunk_info.append(dict(classes=coffs, WT=WT, q_real=q_real,
                               flats=flats, tempcols=tempcols))

    # pass 2: assemble batches. Chunk cc's batch = its grids + the reorder of
    # chunk cc-1 (reading the scratch half (cc-1)%2 that lives inside the
    # table tile at col TBL + half*WTmax); chunk 3's reorder is a final
    # standalone gather.
    WTmax = max(ci["WT"] for ci in chunk_info)

    def reorder_flat(cc):
        base = TBL + (cc % 2) * WTmax
        out = []
        for c in range(N_CORES):
            R = np.full((G, GW), base, np.int32)
            tc_ = chunk_info[cc]["tempcols"][c]
            for g in range(G):
                jj = np.arange(GW_REAL) + g * GW_REAL
                R[g, :GW_REAL] = base + tc_[jj]
            out.append(R)
        return out

    plan = []
    SW = 0
    per_core_idx = [[] for _ in range(N_CORES)]
    for cc in range(N_CHUNKS):
        ci = chunk_info[cc]
        ro_off = ci["q_real"] if cc > 0 else None
        qb = ci["q_real"] + (GW if cc > 0 else 0)
        QB = _round_up(qb, 32)
        rflats = reorder_flat(cc - 1) if cc > 0 else None
        chunk_cols = QB // 16
        for c in range(N_CORES):
            parts = [ci["flats"][c]]
            if cc > 0:
                parts.append(rflats[c])
            if QB > qb:
                parts.append(np.full((G, QB - qb), DUMMY, np.int32))
            blob = _wrap_idx(np.concatenate(parts, axis=1))
            assert blob.shape[1] == chunk_cols
            per_core_idx[c].append(blob)
        plan.append(dict(classes=ci["classes"], WT=ci["WT"], Q=QB,
                         ro_off=ro_off, col_base=SW))
        SW += chunk_cols
    # final standalone reorder (chunk 3's scratch)
    rflats = reorder_flat(N_CHUNKS - 1)
    for c in range(N_CORES):
        per_core_idx[c].append(_wrap_idx(rflats[c]))
    final_ro_base = SW
    SW += GW // 16

    idx_all = []
    for c in range(N_CORES):
        idx_all.append(np.concatenate(per_core_idx[c], axis=1))
        assert idx_all[c].shape == (128, SW)
    meta = dict(WTmax=WTmax, final_ro_base=final_ro_base)
    return plan, idx_all, SW, meta


def build_program(plan, SW, meta):
    nc = bacc.Bacc("TRN2", target_bir_lowering=False, debug=False, num_devices=1)
    f32 = mybir.dt.float32
    xin = nc.dram_tensor("xin", [16, N_NODES], f32, kind="ExternalInput")
    xself = nc.dram_tensor("xself", [16, G * GW], f32, kind="ExternalInput")
    idxin = nc.dram_tensor("idxall", [128, SW], mybir.dt.int16, kind="ExternalInput")
    fin = nc.dram_tensor("fpp", [128, 3 * GW], f32, kind="ExternalInput")
    outd = nc.dram_tensor("out", [16, G * GW], f32, kind="ExternalOutput")

    WTmax = meta["WTmax"]
    TBL2 = TBL + 2 * WTmax  # table + double-buffered scratch halves
    assert TBL2 <= 32768
    slab_max = max(max(p["Q"] for p in plan), 3 * GW)

    with tile.TileContext(nc) as tc:
        with (
            tc.tile_pool(name="fixp", bufs=1) as fixp,
            tc.tile_pool(name="slabp", bufs=1) as slabp,
        ):
            idxs = fixp.tile([128, SW], mybir.dt.int16)
            nc.sync.dma_start(out=idxs[:], in_=idxin[:])
            selfb = fixp.tile([128, GW], f32)
            for g in range(G):
                nc.sync.dma_start(out=selfb[16 * g:16 * (g + 1), :],
                                  in_=xself[:, GW * g:GW * (g + 1)])
            acc = fixp.tile([128, GW], f32)
            table = fixp.tile([128, TBL2], f32)  # persistent across chunks
            nc.vector.memset(table[:, CHUNK:TBL], 0.0)  # dummy zero col

            for cc in range(N_CHUNKS):
                p = plan[cc]
                tb = TBL + (cc % 2) * WTmax  # this chunk's scratch half
                for g in range(G):
                    nc.sync.dma_start(
                        out=table[16 * g:16 * (g + 1), 0:CHUNK],
                        in_=xin[:, CHUNK * cc:CHUNK * (cc + 1)])
                nc.vector.memset(table[:, tb:tb + p["WT"]], 0.0)
                # one gather: this chunk's class grids + reorder of chunk cc-1
                slab = slabp.tile([128, slab_max], f32, tag="slab")
                cbase = p["col_base"]
                nc.gpsimd.ap_gather(
                    slab[:, :p["Q"]], table[:, :TBL2],
                    idxs[:, cbase:cbase + p["Q"] // 16],
                    channels=128, num_elems=TBL2, d=1, num_idxs=p["Q"])
                if cc == 1:
                    nc.vector.tensor_copy(
                        out=acc[:], in_=slab[:, p["ro_off"]:p["ro_off"] + GW])
                elif cc > 1:
                    nc.vector.tensor_add(
                        out=acc[:], in0=acc[:],
                        in1=slab[:, p["ro_off"]:p["ro_off"] + GW])
                for (K, npad, offK, soff) in p["classes"]:
                    nc.vector.tensor_reduce(
                        table[:, tb + offK:tb + offK + npad],
                        slab[:, soff:soff + npad * K].rearrange(
                            "p (n k) -> p n k", k=K),
                        axis=mybir.AxisListType.X, op=mybir.AluOpType.add)
            # final standalone reorder for chunk 3's scratch (slab slot reused)
            ro = slabp.tile([128, slab_max], f32, tag="slab")
            rb = meta["final_ro_base"]
            nc.gpsimd.ap_gather(
                ro[:, :GW], table[:, :TBL2], idxs[:, rb:rb + GW // 16],
                channels=128, num_elems=TBL2, d=1, num_idxs=GW)
            nc.vector.tensor_add(out=acc[:], in0=acc[:], in1=ro[:, :GW])

            # fbuf reuses the slab slot (WAR ordering handled by Tile);
            # loaded only for the final combine
            fbuf = slabp.tile([128, slab_max], f32, tag="slab")
            nc.sync.dma_start(out=fbuf[:, :3 * GW], in_=fin[:])
            outb = selfb  # combine in place; selfb not needed afterwards
            nc.vector.tensor_mul(out=outb[:], in0=selfb[:], in1=fbuf[:, 0:GW])
            nc.vector.tensor_mul(out=acc[:], in0=acc[:], in1=fbuf[:, GW:2 * GW])
            nc.vector.tensor_add(out=outb[:], in0=outb[:], in1=acc[:])
            nc.vector.tensor_add(out=outb[:], in0=outb[:],
                                 in1=fbuf[:, 2 * GW:3 * GW])
            for g in range(G):
                nc.sync.dma_start(out=outd[:, GW * g:GW * (g + 1)],
                                  in_=outb[16 * g:16 * (g + 1), :])
    nc.compile()
    return nc


def _per_core_slices(xfull):
    """xfull [16, N] -> per-core xself [16, G*GW] group-major padded."""
    out = []
    for c in range(N_CORES):
        xs = np.zeros((16, G * GW), np.float32)
        for g in range(G):
            lo = NPC * c + GW_REAL * g
            sz = min(GW_REAL, NPC - GW_REAL * g)
            if sz > 0:
                xs[:, GW * g:GW * g + sz] = xfull[:, lo:lo + sz]
        out.append(xs)
    return out


def _assemble(outs):
    """per-core out [16, G*GW] -> xfull [16, N]."""
    xf = np.empty((16, N_NODES), np.float32)
    for c in range(N_CORES):
        o = outs[c]
        for g in range(G):
            lo = NPC * c + GW_REAL * g
            sz = min(GW_REAL, NPC - GW_REAL * g)
            if sz > 0:
                xf[:, lo:lo + sz] = o[:, GW * g:GW * g + sz]
    return xf


def _fpp(f1, f2, b):
    """per-node f1/f2 [N] + scalar bias -> per-core [128, 3*GW] tensors."""
    out = []
    for c in range(N_CORES):
        t = np.zeros((128, 3 * GW), np.float32)
        for g in range(G):
            lo = NPC * c + GW_REAL * g
            sz = min(GW_REAL, NPC - GW_REAL * g)
            rows = slice(16 * g, 16 * (g + 1))
            if sz > 0:
                t[rows, 0:sz] = np.broadcast_to(f1[lo:lo + sz], (16, sz))
                t[rows, GW:GW + sz] = np.broadcast_to(f2[lo:lo + sz], (16, sz))
            t[rows, 2 * GW:3 * GW] = b
        out.append(t)
    return out


_CACHE = {}


def kernel(x, edge_index, alpha1, gamma, bias):
    x = np.asarray(x, dtype=np.float32)
    edge_index = np.asarray(edge_index)
    alpha1 = np.asarray(alpha1, dtype=np.float64)
    gamma = np.asarray(gamma, dtype=np.float64)
    bias = np.asarray(bias, dtype=np.float64)
    n_layers = alpha1.shape[0]

    key = hash(edge_index.tobytes())
    if key not in _CACHE:
        plan, idx_all, SW, meta = make_plan(edge_index)
        nc = build_program(plan, SW, meta)
        _CACHE[key] = (idx_all, nc)
    idx_all, nc = _CACHE[key]

    src = np.asarray(edge_index[0], dtype=np.int64)
    deg = np.bincount(src, minlength=N_NODES).astype(np.float64)
    with np.errstate(divide="ignore"):
        logdeg = np.log(deg)

    xcur = x
    for l in range(n_layers):
        a1 = alpha1[l].reshape(())
        dp = 1.0 / (1.0 + np.exp(-gamma[l].reshape(())))
        self_w = np.exp(a1)
        neigh_w = self_w * np.tanh(a1)
        b = bias[l].reshape(())
        f1 = (self_w * np.exp(dp * logdeg)).astype(np.float32)
        f2 = (neigh_w * np.exp((dp - 1.0) * logdeg)).astype(np.float32)
        fs = _fpp(f1, f2, np.float32(b))
        xselfs = _per_core_slices(xcur)
        in_maps = [
            {"xin": np.ascontiguousarray(xcur), "xself": xselfs[c],
             "idxall": idx_all[c], "fpp": fs[c]}
            for c in range(N_CORES)
        ]
        last_exc = None
        for attempt in range(3):
            try:
                res = run_bass_kernel_spmd(nc, in_maps,
                                           core_ids=list(range(N_CORES)))
                break
            except Exception as e:  # transient NRT/device hiccups
                last_exc = e
                import time as _t
                _t.sleep(5.0 * (attempt + 1))
        else:
            raise last_exc
        xcur = _assemble([r["out"] for r in res.results])
    return xcur


# revision 16
# speedup vs baseline: 1.0417x; 1.0417x over previous
"""DGMRF 2-layer GNN message passing on 8 TRN2 NeuronCores.

Strategy (per layer, per core):
  - Nodes sharded by dst: core c owns dst nodes [12500c, 12500(c+1)).
  - Aggregation aggr[:, d] = sum_{e: dst=d} x[:, src_e] done on-device via
    GPSIMD ap_gather (SBUF free-dim gather, per-16-partition-group index
    lists) + DVE tensor_reduce over per-dst slot grids.
  - x is chunked into 4 source chunks of 25000 nodes (int16 index limit);
    per chunk, per exact-degree class K, a padded [Npad, K] grid of
    chunk-local src indices is gathered and reduced; per-chunk results are
    reordered into natural dst order with a second (small) ap_gather and
    accumulated.
  - Per-node factors f1/f2 (degree weights x layer scalars) are computed on
    host; out = f1*x_self + f2*aggr + bias.
  - One compiled SPMD program, run once per layer; host re-assembles the
    full x between layers (the cross-core "halo exchange").
"""
import numpy as np

import concourse.bass as bass
import concourse.mybir as mybir
import concourse.tile as tile
from concourse import bacc
from concourse.bass_utils import run_bass_kernel_spmd

N_NODES = 100000
N_CORES = 8
NPC = N_NODES // N_CORES          # 12500 dst nodes per core
G = 8                             # partition groups per core
GW_REAL = 1563                    # dst nodes per group (last group short)
GW = 1568                         # padded group width (mult of 16 for idx wrap)
NPC_PAD = G * GW_REAL             # 12504
CHUNK = 25000
N_CHUNKS = N_NODES // CHUNK       # 4
TBL = CHUNK + 1                   # table width incl zero dummy col
DUMMY = CHUNK                     # dummy (zero) column index


def _round_up(x, m):
    return (x + m - 1) // m * m


def _node_group(j):
    """group id for core-local padded dst position j (0..NPC_PAD)."""
    return np.minimum(j // GW_REAL, G - 1)


def _wrap_idx(flat):
    """[G, Q] flat per-group index lists -> wrapped [128, Q//16] int16.

    ap_gather consumes per-group lists interleaved: list position q lives at
    partition 16g + q%16, column q//16."""
    Gn, Q = flat.shape
    assert Q % 16 == 0
    w = flat.reshape(Gn, Q // 16, 16).transpose(0, 2, 1)  # [G, 16, Q//16]
    return w.reshape(Gn * 16, Q // 16).astype(np.int16)


def make_plan(edge_index):
    """Host-side planning: class structure (global) + per-core index data."""
    src = np.asarray(edge_index[0], dtype=np.int64)
    dst = np.asarray(edge_index[1], dtype=np.int64)
    E = src.shape[0]

    chunk_of = src // CHUNK
    # per (dst, chunk) degree
    cnt = np.bincount(dst * N_CHUNKS + chunk_of, minlength=N_NODES * N_CHUNKS)
    cnt = cnt.reshape(N_NODES, N_CHUNKS)

    # edges sorted by (dst, chunk): per (dst, chunk) contiguous src runs
    order = np.lexsort((chunk_of, dst))
    s_src_local = (src[order] % CHUNK).astype(np.int32)
    starts = np.zeros(N_NODES * N_CHUNKS + 1, np.int64)
    np.cumsum(cnt.reshape(-1), out=starts[1:])

    # core-local padded dst table: pos (c, j) -> node id or -1 (pad)
    # groups are contiguous slices of width GW_REAL
    core_node = np.full((N_CORES, NPC_PAD), -1, np.int64)
    for c in range(N_CORES):
        core_node[c, :NPC] = np.arange(NPC * c, NPC * (c + 1))
    grp_of_j = _node_group(np.arange(NPC_PAD))

    # pass 1 per chunk: class structure (global across cores/groups),
    # per-core grid flats and tempcol maps
    chunk_info = []
    for cc in range(N_CHUNKS):
        deg = np.zeros((N_CORES, NPC_PAD), np.int64)
        valid = core_node >= 0
        deg[valid] = cnt[core_node[valid], cc]

        kmax = int(deg.max())
        # global per-(core,group,K) counts
        counts = np.zeros((N_CORES, G, kmax + 1), np.int64)
        for c in range(N_CORES):
            key = grp_of_j * (kmax + 1) + deg[c]
            bc = np.bincount(key, minlength=G * (kmax + 1))
            counts[c] = bc.reshape(G, kmax + 1)
        gmax = counts.max(axis=(0, 1))  # [kmax+1] max members per group

        # kept classes: merge small ones upward
        kept = []
        for K in range(1, kmax + 1):
            if gmax[K] == 0:
                continue
            kept.append(K)
        # merge: keep K if its max count >= 48, else merge into next kept
        merged = []
        for K in kept:
            merged.append(K)
        # assignment class of each deg d (>0): smallest kept >= d
        keep_mask = [False] * (kmax + 2)
        sel = []
        for K in kept:
            if gmax[K] >= 48 or K == kept[-1]:
                sel.append(K)
        if not sel or sel[-1] < kmax:
            sel.append(kmax)
        sel = sorted(set(sel))
        cls_of = np.zeros(kmax + 1, np.int64)  # deg -> class K (0 for deg 0)
        si = 0
        for d in range(1, kmax + 1):
            while sel[si] < d:
                si += 1
            cls_of[d] = sel[si]

        # per class: Npad (global), temp offset
        cls_list = []
        off = 0
        npad_of = {}
        for K in sel:
            mx = 0
            for c in range(N_CORES):
                cdeg = deg[c]
                mask = (cdeg > 0) & (cls_of[cdeg] == K)
                bc = np.bincount(grp_of_j[mask], minlength=G)
                mx = max(mx, int(bc.max()))
            # classes are merged into one gather per chunk, so only the batch
            # total needs idx alignment; npad just needs a zero pad cell
            npad = _round_up(mx + 1, 2)
            npad_of[K] = (npad, off)
            cls_list.append((K, npad, off))
            off += npad
        WT = off
        zero_cell = cls_list[0][2] + cls_list[0][1] - 1  # a guaranteed 0 cell

        q_real = sum(npad * K for K, npad, _ in cls_list)

        flats = []     # per core: [G, q_real] grid content (table cols)
        tempcols = []  # per core: [NPC_PAD] scratch col of each dst
        for c in range(N_CORES):
            cdeg = deg[c]
            flat_parts = []
            tempcol = np.full(NPC_PAD, zero_cell, np.int64)
            for K, npad, offK in cls_list:
                mask = (cdeg > 0) & (cls_of[cdeg] == K)
                js = np.nonzero(mask)[0]
                gs = grp_of_j[js]
                # rank via cumcount per group (js ascending => natural order)
                r = np.zeros(len(js), np.int64)
                for g in range(G):
                    m = gs == g
                    r[m] = np.arange(m.sum())
                tempcol[js] = offK + r
                # fill srcs: node n, chunk cc: run starts[n*4+cc], length deg
                nodes = core_node[c, js]
                st = starts[nodes * N_CHUNKS + cc]
                dg = cdeg[js]
                A = np.full((G * npad, K), DUMMY, np.int32)
                if len(js):
                    kk = np.arange(K)[None, :]
                    valid = kk < dg[:, None]
                    pos = np.minimum(st[:, None] + kk, len(s_src_local) - 1)
                    vals = s_src_local[pos]
                    A[gs * npad + r] = np.where(valid, vals, DUMMY)
                flat_parts.append(A.reshape(G, npad * K))
            flats.append(np.concatenate(flat_parts, axis=1))
            tempcols.append(tempcol)

        coffs = []
        so = 0
        for K, npad, offK in cls_list:
            coffs.append((K, npad, offK, so))
            so += npad * K
        assert so == q_real
        chunk_info.append(dict(classes=coffs, WT=WT, q_real=q_real,
                               flats=flats, tempcols=tempcols))

    # pass 2: assemble batches. Chunk cc's batch = its grids + the reorder of
    # chunk cc-1 (reading the scratch half (cc-1)%2 that lives inside the
    # table tile at col TBL + half*WTmax); chunk 3's reorder is a final
    # standalone gather.
    WTmax = max(ci["WT"] for ci in chunk_info)

    def reorder_flat(cc):
        base = TBL + (cc % 2) * WTmax
        out = []
        for c in range(N_CORES):
            R = np.full((G, GW), base, np.int32)
            tc_ = chunk_info[cc]["tempcols"][c]
            for g in range(G):
                jj = np.arange(GW_REAL) + g * GW_REAL
                R[g, :GW_REAL] = base + tc_[jj]
            out.append(R)
        return out

    plan = []
    SW = 0
    per_core_idx = [[] for _ in range(N_CORES)]
    for cc in range(N_CHUNKS):
        ci = chunk_info[cc]
        ro_off = ci["q_real"] if cc > 0 else None
        qb = ci["q_real"] + (GW if cc > 0 else 0)
        QB = _round_up(qb, 32)
        rflats = reorder_flat(cc - 1) if cc > 0 else None
        chunk_cols = QB // 16
        for c in range(N_CORES):
            parts = [ci["flats"][c]]
            if cc > 0:
                parts.append(rflats[c])
            if QB > qb:
                parts.append(np.full((G, QB - qb), DUMMY, np.int32))
            blob = _wrap_idx(np.concatenate(parts, axis=1))
            assert blob.shape[1] == chunk_cols
            per_core_idx[c].append(blob)
        plan.append(dict(classes=ci["classes"], WT=ci["WT"], Q=QB,
                         ro_off=ro_off, col_base=SW))
        SW += chunk_cols
    # final standalone reorder (chunk 3's scratch)
    rflats = reorder_flat(N_CHUNKS - 1)
    for c in range(N_CORES):
        per_core_idx[c].append(_wrap_idx(rflats[c]))
    final_ro_base = SW
    SW += GW // 16

    idx_all = []
    for c in range(N_CORES):
        idx_all.append(np.concatenate(per_core_idx[c], axis=1))
        assert idx_all[c].shape == (128, SW)
    meta = dict(WTmax=WTmax, final_ro_base=final_ro_base)
    return plan, idx_all, SW, meta


def build_program(plan, SW, meta):
    nc = bacc.Bacc("TRN2", target_bir_lowering=False, debug=False, num_devices=1)
    f32 = mybir.dt.float32
    xin = nc.dram_tensor("xin", [16, N_NODES], f32, kind="ExternalInput")
    xself = nc.dram_tensor("xself", [16, G * GW], f32, kind="ExternalInput")
    idxin = nc.dram_tensor("idxall", [128, SW], mybir.dt.int16, kind="ExternalInput")
    fin = nc.dram_tensor("fpp", [128, 3 * GW], f32, kind="ExternalInput")
    outd = nc.dram_tensor("out", [16, G * GW], f32, kind="ExternalOutput")

    WTmax = meta["WTmax"]
    TBL2 = TBL + 2 * WTmax  # table + double-buffered scratch halves
    assert TBL2 <= 32768
    slab_max = max(max(p["Q"] for p in plan), 3 * GW)

    with tile.TileContext(nc) as tc:
        with (
            tc.tile_pool(name="fixp", bufs=1) as fixp,
            tc.tile_pool(name="slabp", bufs=1) as slabp,
        ):
            idxs = fixp.tile([128, SW], mybir.dt.int16)
            nc.sync.dma_start(out=idxs[:], in_=idxin[:])
            selfb = fixp.tile([128, GW], f32)
            for g in range(G):
                nc.sync.dma_start(out=selfb[16 * g:16 * (g + 1), :],
                                  in_=xself[:, GW * g:GW * (g + 1)])
            acc = fixp.tile([128, GW], f32)
            table = fixp.tile([128, TBL2], f32)  # persistent across chunks
            nc.vector.memset(table[:, CHUNK:TBL], 0.0)  # dummy zero col

            for cc in range(N_CHUNKS):
                p = plan[cc]
                tb = TBL + (cc % 2) * WTmax  # this chunk's scratch half
                for g in range(G):
                    nc.sync.dma_start(
                        out=table[16 * g:16 * (g + 1), 0:CHUNK],
                        in_=xin[:, CHUNK * cc:CHUNK * (cc + 1)])
                nc.vector.memset(table[:, tb:tb + p["WT"]], 0.0)
                # one gather: this chunk's class grids + reorder of chunk cc-1
                slab = slabp.tile([128, slab_max], f32, tag="slab")
                cbase = p["col_base"]
                nc.gpsimd.ap_gather(
                    slab[:, :p["Q"]], table[:, :TBL2],
                    idxs[:, cbase:cbase + p["Q"] // 16],
                    channels=128, num_elems=TBL2, d=1, num_idxs=p["Q"])
                if cc == 1:
                    nc.vector.tensor_copy(
                        out=acc[:], in_=slab[:, p["ro_off"]:p["ro_off"] + GW])
                elif cc > 1:
                    nc.vector.tensor_add(
                        out=acc[:], in0=acc[:],
                        in1=slab[:, p["ro_off"]:p["ro_off"] + GW])
                for (K, npad, offK, soff) in p["classes"]:
                    nc.vector.tensor_reduce(
                        table[:, tb + offK:tb + offK + npad],
                        slab[:, soff:soff + npad * K].rearrange(
                            "p (n k) -> p n k", k=K),
                        axis=mybir.AxisListType.X, op=mybir.AluOpType.add)
            # final standalone reorder for chunk 3's scratch (slab slot reused)
            ro = slabp.tile([128, slab_max], f32, tag="slab")
            rb = meta["final_ro_base"]
            nc.gpsimd.ap_gather(
                ro[:, :GW], table[:, :TBL2], idxs[:, rb:rb + GW // 16],
                channels=128, num_elems=TBL2, d=1, num_idxs=GW)
            nc.vector.tensor_add(out=acc[:], in0=acc[:], in1=ro[:, :GW])

            # fbuf reuses the slab slot (WAR ordering handled by Tile);
            # loaded only for the final combine
            fbuf = slabp.tile([128, slab_max], f32, tag="slab")
            nc.sync.dma_start(out=fbuf[:, :3 * GW], in_=fin[:])
            outb = selfb  # combine in place; selfb not needed afterwards
            nc.vector.tensor_mul(out=outb[:], in0=selfb[:], in1=fbuf[:, 0:GW])
            nc.vector.tensor_mul(out=acc[:], in0=acc[:], in1=fbuf[:, GW:2 * GW])
            nc.vector.tensor_add(out=outb[:], in0=outb[:], in1=acc[:])
            nc.vector.tensor_add(out=outb[:], in0=outb[:],
                                 in1=fbuf[:, 2 * GW:3 * GW])
            for g in range(G):
                nc.sync.dma_start(out=outd[:, GW * g:GW * (g + 1)],
                                  in_=outb[16 * g:16 * (g + 1), :])
    nc.compile()
    return nc


def _per_core_slices(xfull):
    """xfull [16, N] -> per-core xself [16, G*GW] group-major padded."""
    out = []
    for c in range(N_CORES):
        xs = np.zeros((16, G * GW), np.float32)
        for g in range(G):
            lo = NPC * c + GW_REAL * g
            sz = min(GW_REAL, NPC - GW_REAL * g)
            if sz > 0:
                xs[:, GW * g:GW * g + sz] = xfull[:, lo:lo + sz]
        out.append(xs)
    return out


def _assemble(outs):
    """per-core out [16, G*GW] -> xfull [16, N]."""
    xf = np.empty((16, N_NODES), np.float32)
    for c in range(N_CORES):
        o = outs[c]
        for g in range(G):
            lo = NPC * c + GW_REAL * g
            sz = min(GW_REAL, NPC - GW_REAL * g)
            if sz > 0:
                xf[:, lo:lo + sz] = o[:, GW * g:GW * g + sz]
    return xf


def _fpp(f1, f2, b):
    """per-node f1/f2 [N] + scalar bias -> per-core [128, 3*GW] tensors."""
    out = []
    for c in range(N_CORES):
        t = np.zeros((128, 3 * GW), np.float32)
        for g in range(G):
            lo = NPC * c + GW_REAL * g
            sz = min(GW_REAL, NPC - GW_REAL * g)
            rows = slice(16 * g, 16 * (g + 1))
            if sz > 0:
                t[rows, 0:sz] = np.broadcast_to(f1[lo:lo + sz], (16, sz))
                t[rows, GW:GW + sz] = np.broadcast_to(f2[lo:lo + sz], (16, sz))
            t[rows, 2 * GW:3 * GW] = b
        out.append(t)
    return out


_CACHE = {}


def kernel(x, edge_index, alpha1, gamma, bias):
    x = np.asarray(x, dtype=np.float32)
    edge_index = np.asarray(edge_index)
    alpha1 = np.asarray(alpha1, dtype=np.float64)
    gamma = np.asarray(gamma, dtype=np.float64)
    bias = np.asarray(bias, dtype=np.float64)
    n_layers = alpha1.shape[0]

    key = hash(edge_index.tobytes())
    if key not in _CACHE:
        plan, idx_all, SW, meta = make_plan(edge_index)
        nc = build_program(plan, SW, meta)
        _CACHE[key] = (idx_all, nc)
    idx_all, nc = _CACHE[key]

    src = np.asarray(edge_index[0], dtype=np.int64)
    deg = np.bincount(src, minlength=N_NODES).astype(np.float64)
    with np.errstate(divide="ignore"):
        logdeg = np.log(deg)

    xcur = x
    for l in range(n_layers):
        a1 = alpha1[l].reshape(())
        dp = 1.0 / (1.0 + np.exp(-gamma[l].reshape(())))
        self_w = np.exp(a1)
        neigh_w = self_w * np.tanh(a1)
        b = bias[l].reshape(())
        f1 = (self_w * np.exp(dp * logdeg)).astype(np.float32)
        f2 = (neigh_w * np.exp((dp - 1.0) * logdeg)).astype(np.float32)
        fs = _fpp(f1, f2, np.float32(b))
        xselfs = _per_core_slices(xcur)
        in_maps = [
            {"xin": np.ascontiguousarray(xcur), "xself": xselfs[c],
             "idxall": idx_all[c], "fpp": fs[c]}
            for c in range(N_CORES)
        ]
        last_exc = None
        for attempt in range(4):
            try:
                res = run_bass_kernel_spmd(nc, in_maps,
                                           core_ids=list(range(N_CORES)))
                break
            except Exception as e:  # transient NRT/device hiccups; a wedged
                last_exc = e        # device can take minutes to recover
                import time as _t
                _t.sleep(20.0 * (attempt + 1))
        else:
            raise last_exc
        xcur = _assemble([r["out"] for r in res.results])
    return xcur
